# revision 24
# baseline (speedup 1.0000x reference)
"""Trainium2 Bass kernel for nn_Attention_3710851743764.

Full attention block: qkv proj -> per-head RMSNorm(q,k) -> RoPE -> GQA
attention (16 q heads, 4 kv heads, S=2048, D=128) -> out proj.

Sharding: 8 cores = 2 (batch) x 4 (kv-head groups). Each core computes its
batch's qkv for its group (4 q heads + 1 kv head), full attention for those
heads, and a partial output projection (its 512 wo columns); the host sums
the 4 partials per batch.

Dataflow is fully "transposed" (features on partitions, tokens on free):
  qkvT[f,t]   = mm(lhsT=wqkvT[d,f], rhs=xT[d,t])            accumulated over d
  ssq[c,t]    = mm(lhsT=esel[:,c,:], rhs=square(qkvT_c))     (RMS factors)
  rot[d',t]   = mm(lhsT=P_rot, rhs=qn)                       (RoPE pair swap)
  scoresT[s,t]= mm(lhsT=kT[:,s-blk], rhs=qT_h)               per 128-s block
  pT          = exp(scoresT)          (no max subtraction: |score|<=sqrt(128))
  attnT[d,t]  = mm(lhsT=v[s-blk,d], rhs=pT)                  accumulated over s
  denom[h,t]  = mm(lhsT=esel[:,h,:4], rhs=pT)                accumulated
  out[t,o]    = mm(lhsT=attnT_n[f,t-blk], rhs=woT[f,o])      accumulated over f

All matmuls run in float32r (tf32-like, full PE rate).
"""

import sys

sys.path.insert(0, "/opt/trn_rl_repo")

import numpy as np

import concourse.bass as bass
import concourse.tile as tile
from concourse import bacc, mybir
from concourse import bass_utils

F32 = mybir.dt.float32
F32R = mybir.dt.float32r
AF = mybir.ActivationFunctionType
OP = mybir.AluOpType

DIM = 2048
N_HEADS = 16
N_KV = 4
HEAD_DIM = 128
B = 2
S = 2048
EPS = float(np.finfo(np.float32).eps)
GQ = N_HEADS // N_KV          # q heads per group = 4
GF = GQ * HEAD_DIM            # group q features = 512
P = 128
KC = DIM // P                 # 16 contraction chunks for projections
TC = 4                        # token chunks of 512
SC = S // P                   # 16 key chunks of 128
NF = GF + 2 * HEAD_DIM        # 768 qkv features per group
FC = NF // P                  # 6 feature chunks

_CACHED_NC = None


def build_nc():
    """Build the single-core Bass program (same program for all 8 cores)."""
    nc = bacc.Bacc("TRN2", target_bir_lowering=False, debug=False,
                   num_devices=8)

    xT_d = nc.dram_tensor("xT", [TC, P, KC, 512], F32R,
                          kind="ExternalInput").ap()
    wqkvT_d = nc.dram_tensor("wqkvT", [P, KC, NF], F32R,
                             kind="ExternalInput").ap()
    woT_d = nc.dram_tensor("woT", [HEAD_DIM, GQ, DIM], F32R,
                           kind="ExternalInput").ap()
    cosT_d = nc.dram_tensor("cosT", [HEAD_DIM, S], F32,
                            kind="ExternalInput").ap()
    sinT_d = nc.dram_tensor("sinT", [HEAD_DIM, S], F32,
                            kind="ExternalInput").ap()
    normw_d = nc.dram_tensor("normw", [P, 2], F32, kind="ExternalInput").ap()
    prot_d = nc.dram_tensor("prot", [P, P], F32R, kind="ExternalInput").ap()
    ident_d = nc.dram_tensor("ident", [P, P], F32R, kind="ExternalInput").ap()
    esel_d = nc.dram_tensor("esel", [P, 5, 5], F32R,
                            kind="ExternalInput").ap()
    out_d = nc.dram_tensor("out", [SC, P, TC, 512], F32,
                           kind="ExternalOutput").ap()

    with tile.TileContext(nc) as tc:
        with (
            tc.tile_pool(name="consts", bufs=1) as cp,
        ):
            dramp = tc.alloc_tile_pool(name="dram_scratch", bufs=1,
                                       space="DRAM")
            rfac_dr = dramp.tile([5, S], F32, name="rfac_dr")
            rd_dr = [dramp.tile([4, 1024], F32, name=f"rd_dr{i}")
                     for i in range(2)]
            # stage-scoped persistent tensors (manual release for SBUF reuse)
            p1 = tc.alloc_tile_pool(name="p1", bufs=1)   # lives A..C
            qkv_raw = p1.tile([P, 5, S], F32, name="qkv_raw")     # 40KB
            vT_sb = p1.tile([P, S], F32R, name="vT_sb")           # 8KB
            rfac = p1.tile([5, S], F32, name="rfac")

            cos_sb = cp.tile([HEAD_DIM, S], F32, name="cos_sb")
            sin_sb = cp.tile([HEAD_DIM, S], F32, name="sin_sb")
            normw_sb = cp.tile([P, 2], F32, name="normw_sb")
            prot_sb = cp.tile([P, P], F32R, name="prot_sb")
            ident_sb = cp.tile([P, P], F32R, name="ident_sb")
            esel_sb = cp.tile([P, 5, 5], F32R, name="esel_sb")
            eps_sb = cp.tile([P, 1], F32, name="eps_sb")
            zero_sb = cp.tile([P, 1], F32, name="zero_sb")
            nc.vector.memset(eps_sb[:], EPS)
            nc.vector.memset(zero_sb[:], 0.0)
            nc.sync.dma_start(esel_sb[:], esel_d)

            # ---------------- Stage A: qkv projection + squares ----------
            # fc-outer so each weight chunk (lhsT) is reused across the
            # 4 token chunks (amortizes LDWEIGHTS 4x); full xT resident.
            with (
                tc.tile_pool(name="stA", bufs=2) as sa,
                tc.tile_pool(name="wq_pool", bufs=1) as wp,
                tc.tile_pool(name="psA", bufs=4, space="PSUM") as psA,
                tc.tile_pool(name="psSq", bufs=1, space="PSUM") as psSq,
            ):
                wq_sb = wp.tile([P, KC, NF], F32R, name="wq_sb")  # 48KB
                ssq_ps = psSq.tile([5, S], F32, name="ssq_ps")    # 4 banks
                for tcc in range(TC):
                    xt = sa.tile([P, KC, 512], F32R, name="xt")   # 32KB x2
                    step = 2 if tcc == 0 else 4
                    for kc4 in range(0, KC, step):
                        nc.sync.dma_start(xt[:, kc4:kc4 + step, :],
                                          xT_d[tcc, :, kc4:kc4 + step, :])
                        if tcc == 0:
                            nc.sync.dma_start(wq_sb[:, kc4:kc4 + step, :],
                                              wqkvT_d[:, kc4:kc4 + step, :])
                    for fc in range(FC):
                        ps = psA.tile([P, 512], F32, name="qkv_ps")
                        for kc in range(KC):
                            nc.tensor.matmul(
                                ps[:],
                                wq_sb[:, kc, fc * P:(fc + 1) * P],
                                xt[:, kc, :],
                                start=(kc == 0), stop=(kc == KC - 1))
                        tsl = slice(tcc * 512, (tcc + 1) * 512)
                        if fc < 5:
                            sq = sa.tile([P, 512], F32R, name="sq")
                            nc.scalar.activation(sq[:], ps[:], AF.Square,
                                                 bias=zero_sb[:])
                            nc.tensor.matmul(
                                ssq_ps[:, tsl], esel_sb[:, fc, :],
                                sq[:],
                                start=(fc == 0), stop=(fc == 4),
                                skip_group_check=True)
                            nc.vector.tensor_copy(qkv_raw[:, fc, tsl], ps[:])
                        else:
                            nc.vector.tensor_copy(vT_sb[:, tsl], ps[:])
                nc.sync.dma_start(cos_sb[:], cosT_d)
                nc.sync.dma_start(sin_sb[:], sinT_d)
                nc.sync.dma_start(normw_sb[:], normw_d)
                nc.sync.dma_start(prot_sb[:], prot_d)
                nc.sync.dma_start(ident_sb[:], ident_d)
                # Stage B: rms factors  rfac = 1/sqrt(ssq/128 + eps)
                std = wp.tile([5, S], F32, name="std")
                nc.scalar.activation(std[:], ssq_ps[:], AF.Sqrt,
                                     scale=1.0 / HEAD_DIM, bias=eps_sb[0:5, :])
                nc.vector.reciprocal_approx_fast(rfac[:], std[:])
                nc.gpsimd.dma_start(rfac_dr[:], rfac[:])

            # ---------------- Stage C: normalize + rope + v transpose ----
            p2 = tc.alloc_tile_pool(name="p2", bufs=1, side="right")  # C..D
            qk_sb = [p2.tile([P, S], F32R, name=f"qk_sb{i}")
                     for i in range(5)]                           # 40KB
            v_sb = p2.tile([P, SC, HEAD_DIM], F32R, name="v_sb")  # 8KB
            # scores psum allocated BEFORE stage C so its banks are not
            # WAR-blocked on C's rope/transpose psum (lets stage D start
            # while C's DVE tail is still running)
            psS = tc.alloc_tile_pool(name="psS", bufs=4, space="PSUM")
            with (
                tc.tile_pool(name="stC", bufs=2) as sc_pool,
                tc.tile_pool(name="psC", bufs=2, space="PSUM") as psC,
            ):
                for fc in (4, 0, 1, 2, 3):
                    rb = sc_pool.tile([P, S], F32, name="rb")
                    nc.gpsimd.dma_start(
                        rb[:], rfac_dr[fc:fc + 1, :].to_broadcast((P, S)))
                    qn = sc_pool.tile([P, S], F32R, name="qn")
                    wcol = 0 if fc < 4 else 1
                    nc.vector.scalar_tensor_tensor(
                        qn[:], qkv_raw[:, fc, :],
                        normw_sb[:, wcol:wcol + 1], rb[:],
                        op0=OP.mult, op1=OP.mult)
                    for tcc in range(TC):
                        tsl = slice(tcc * 512, (tcc + 1) * 512)
                        rot_ps = psC.tile([P, 512], F32, name="rot_ps")
                        nc.tensor.matmul(rot_ps[:], prot_sb[:], qn[:, tsl],
                                         start=True, stop=True)
                        rs = sc_pool.tile([P, 512], F32, name="rs")
                        nc.vector.tensor_mul(rs[:], rot_ps[:], sin_sb[:, tsl])
                        qc = sc_pool.tile([P, 512], F32, name="qc")
                        nc.gpsimd.tensor_mul(qc[:], qn[:, tsl].bitcast(F32),
                                             cos_sb[:, tsl])
                        aeng = nc.vector if fc in (4, 0) else nc.gpsimd
                        aeng.tensor_add(qk_sb[fc][:, tsl], qc[:], rs[:])
                for scc in range(SC):
                    vt_ps = psC.tile([P, P], F32R, name="vt_ps")
                    nc.tensor.transpose(
                        vt_ps[:], vT_sb[:, scc * P:(scc + 1) * P],
                        ident_sb[:])
                    nc.vector.tensor_copy(v_sb[:, scc, :], vt_ps[:])

            p1.release()

            # ---------------- Stage D: attention ------------------------
            p3 = tc.alloc_tile_pool(name="p3", bufs=1)   # lives D..E
            atn_raw = [p3.tile([P, GQ, 1024], F32, name=f"atn_raw{i}")
                       for i in range(2)]                         # 32KB
            atn_n = [p3.tile([P, GQ, 1024], F32R, name=f"atn_n{i}")
                     for i in range(2)]                           # 32KB
            woT_sb = p3.tile([P, GQ, DIM], F32R, name="woT_sb")   # 32KB
            with (
                tc.tile_pool(name="stD", bufs=2) as sd,
                tc.tile_pool(name="ptp", bufs=6) as ptp,
                tc.tile_pool(name="psPV", bufs=1, space="PSUM") as psPV,
                tc.tile_pool(name="psDN", bufs=1, space="PSUM") as psDN,
            ):
                for pair in range(2):
                    po = pair * 1024
                    dn_ps = psDN.tile([4, 1024], F32, name="dn_ps")
                    for h in range(GQ):
                        pv_ps = psPV.tile([P, 1024], F32, name="pv_ps")
                        for scc in range(SC):
                            ksl = qk_sb[4][:, scc * P:(scc + 1) * P]
                            for half in range(2):
                                hs = slice(half * 512, half * 512 + 512)
                                sp = psS.tile([P, 512], F32, name="sp")
                                nc.tensor.matmul(
                                    sp[:], ksl,
                                    qk_sb[h][:, po + half * 512:
                                              po + half * 512 + 512],
                                    start=True, stop=True)
                                pt = ptp.tile([P, 512], F32R, name="pt")
                                nc.scalar.activation(pt[:], sp[:], AF.Exp,
                                                     bias=zero_sb[:])
                                nc.tensor.matmul(
                                    pv_ps[:, hs], v_sb[:, scc, :], pt[:],
                                    start=(scc == 0), stop=(scc == SC - 1))
                                nc.tensor.matmul(
                                    dn_ps[:, hs], esel_sb[:, h, 0:4],
                                    pt[:],
                                    start=(h == 0 and scc == 0),
                                    stop=(h == GQ - 1 and scc == SC - 1),
                                    skip_group_check=True)
                        nc.vector.tensor_copy(atn_raw[pair][:, h, :],
                                              pv_ps[:])
                    rd = sd.tile([4, 1024], F32, name="rd")
                    nc.vector.reciprocal_approx_fast(rd[:], dn_ps[:])
                    nc.gpsimd.dma_start(rd_dr[pair][:], rd[:])
                    for h in range(GQ):
                        rbh = sd.tile([P, 1024], F32, name="rbh")
                        nc.gpsimd.dma_start(
                            rbh[:],
                            rd_dr[pair][h:h + 1, :].to_broadcast((P, 1024)))
                        nc.vector.tensor_mul(atn_n[pair][:, h, :],
                                             atn_raw[pair][:, h, :],
                                             rbh[:])

            psS.release()
            p2.release()

            # ---------------- Stage E: output projection -----------------
            nc.sync.dma_start(woT_sb[:], woT_d)
            with (
                tc.tile_pool(name="stE", bufs=4) as se,
                tc.tile_pool(name="psE", bufs=8, space="PSUM") as psE,
            ):
                for tcc in range(SC):
                    pr = tcc // 8
                    tloc = (tcc % 8) * P
                    ps4 = [psE.tile([P, 512], F32, name="out_ps")
                           for _ in range(TC)]
                    for h in range(GQ):
                        lhs = atn_n[pr][:, h, tloc:tloc + P]
                        for oc in range(TC):
                            nc.tensor.matmul(
                                ps4[oc][:], lhs,
                                woT_sb[:, h, oc * 512:(oc + 1) * 512],
                                start=(h == 0), stop=(h == GQ - 1),
                                skip_group_check=True)
                    ob = se.tile([P, TC, 512], F32, name="ob")
                    for oc in range(TC):
                        nc.vector.tensor_copy(ob[:, oc, :], ps4[oc][:])
                    nc.sync.dma_start(out_d[tcc], ob[:])
            p3.release()

    nc.compile()
    return nc


def make_in_maps(x, wqkv, wo, q_norm_w, k_norm_w, freqs_cos, freqs_sin):
    """Build the 8 per-core input maps. Core c = b*4 + g."""
    x = np.asarray(x, np.float32)
    wqkv = np.asarray(wqkv, np.float32)
    wo = np.asarray(wo, np.float32)
    q_norm_w = np.asarray(q_norm_w, np.float32)
    k_norm_w = np.asarray(k_norm_w, np.float32)
    cosT = np.ascontiguousarray(
        np.asarray(freqs_cos, np.float32)[:, 0, :].T)
    sinT = np.ascontiguousarray(
        np.asarray(freqs_sin, np.float32)[:, 0, :].T)

    normw = np.empty((P, 2), np.float32)
    normw[:, 0] = q_norm_w * np.float32(1.0 / np.sqrt(HEAD_DIM))
    normw[:, 1] = k_norm_w

    prot = np.zeros((P, P), np.float32)
    prot[np.arange(1, P, 2), np.arange(0, P, 2)] = -1.0
    prot[np.arange(0, P, 2), np.arange(1, P, 2)] = 1.0
    ident = np.eye(P, dtype=np.float32)
    esel = np.zeros((P, 5, 5), np.float32)
    for c in range(5):
        esel[:, c, c] = 1.0

    q_size = N_HEADS * HEAD_DIM
    kv_size = N_KV * HEAD_DIM
    in_maps = []
    for b in range(B):
        # [tc, p, kc, u]: xT[kc*128+p, tc*512+u] pre-tiled for 1-run/partition
        xT = np.ascontiguousarray(
            x[b].reshape(TC, 512, KC, P).transpose(0, 3, 2, 1))
        for g in range(N_KV):
            wq = wqkv[g * GF:(g + 1) * GF]
            wk = wqkv[q_size + g * HEAD_DIM:q_size + (g + 1) * HEAD_DIM]
            wv = wqkv[q_size + kv_size + g * HEAD_DIM:
                      q_size + kv_size + (g + 1) * HEAD_DIM]
            wqkvT = np.ascontiguousarray(
                np.concatenate([wq, wk, wv], axis=0).T
                .reshape(KC, P, NF).transpose(1, 0, 2))
            woT = np.ascontiguousarray(
                wo[:, g * GF:(g + 1) * GF].T.reshape(GQ, HEAD_DIM, DIM)
                .transpose(1, 0, 2))
            in_maps.append({
                "xT": xT, "wqkvT": wqkvT, "woT": woT,
                "cosT": cosT, "sinT": sinT, "normw": normw,
                "prot": prot, "ident": ident, "esel": esel,
            })
    return in_maps


def run(in_maps, trace=False):
    global _CACHED_NC
    if _CACHED_NC is None:
        _CACHED_NC = build_nc()
    return bass_utils.run_bass_kernel_spmd(
        _CACHED_NC, in_maps, core_ids=list(range(8)), trace=trace)


def kernel(x, wqkv, wo, q_norm_w, k_norm_w, freqs_cos, freqs_sin):
    in_maps = make_in_maps(x, wqkv, wo, q_norm_w, k_norm_w,
                           freqs_cos, freqs_sin)
    res = run(in_maps, trace=False)
    out = np.zeros((B, S, DIM), np.float32)
    for b in range(B):
        for g in range(N_KV):
            o = res.results[b * N_KV + g]["out"]    # [SC, P, TC, 512]
            out[b] += o.transpose(0, 1, 2, 3).reshape(S, DIM)
    return out


# revision 25
# speedup vs baseline: 1.0290x; 1.0290x over previous
"""Trainium2 Bass kernel for nn_Attention_3710851743764.

Full attention block: qkv proj -> per-head RMSNorm(q,k) -> RoPE -> GQA
attention (16 q heads, 4 kv heads, S=2048, D=128) -> out proj.

Sharding: 8 cores = 2 (batch) x 4 (kv-head groups). Each core computes its
batch's qkv for its group (4 q heads + 1 kv head), full attention for those
heads, and a partial output projection (its 512 wo columns); the host sums
the 4 partials per batch.

Dataflow is fully "transposed" (features on partitions, tokens on free):
  qkvT[f,t]   = mm(lhsT=wqkvT[d,f], rhs=xT[d,t])            accumulated over d
  ssq[c,t]    = mm(lhsT=esel[:,c,:], rhs=square(qkvT_c))     (RMS factors)
  rot[d',t]   = mm(lhsT=P_rot, rhs=qn)                       (RoPE pair swap)
  scoresT[s,t]= mm(lhsT=kT[:,s-blk], rhs=qT_h)               per 128-s block
  pT          = exp(scoresT)          (no max subtraction: |score|<=sqrt(128))
  attnT[d,t]  = mm(lhsT=v[s-blk,d], rhs=pT)                  accumulated over s
  denom[h,t]  = mm(lhsT=esel[:,h,:4], rhs=pT)                accumulated
  out[t,o]    = mm(lhsT=attnT_n[f,t-blk], rhs=woT[f,o])      accumulated over f

All matmuls run in float32r (tf32-like, full PE rate).
"""

import sys

sys.path.insert(0, "/opt/trn_rl_repo")

import numpy as np

import concourse.bass as bass
import concourse.tile as tile
from concourse import bacc, mybir
from concourse import bass_utils

F32 = mybir.dt.float32
F32R = mybir.dt.float32r
AF = mybir.ActivationFunctionType
OP = mybir.AluOpType

DIM = 2048
N_HEADS = 16
N_KV = 4
HEAD_DIM = 128
B = 2
S = 2048
EPS = float(np.finfo(np.float32).eps)
GQ = N_HEADS // N_KV          # q heads per group = 4
GF = GQ * HEAD_DIM            # group q features = 512
P = 128
KC = DIM // P                 # 16 contraction chunks for projections
TC = 4                        # token chunks of 512
SC = S // P                   # 16 key chunks of 128
NF = GF + 2 * HEAD_DIM        # 768 qkv features per group
FC = NF // P                  # 6 feature chunks

_CACHED_NC = None


def build_nc():
    """Build the single-core Bass program (same program for all 8 cores)."""
    nc = bacc.Bacc("TRN2", target_bir_lowering=False, debug=False,
                   num_devices=8)

    xT_d = nc.dram_tensor("xT", [TC, P, KC, 512], F32R,
                          kind="ExternalInput").ap()
    wqkvT_d = nc.dram_tensor("wqkvT", [P, KC, NF], F32R,
                             kind="ExternalInput").ap()
    woT_d = nc.dram_tensor("woT", [HEAD_DIM, GQ, DIM], F32R,
                           kind="ExternalInput").ap()
    cosT_d = nc.dram_tensor("cosT", [HEAD_DIM, S], F32,
                            kind="ExternalInput").ap()
    sinT_d = nc.dram_tensor("sinT", [HEAD_DIM, S], F32,
                            kind="ExternalInput").ap()
    normw_d = nc.dram_tensor("normw", [P, 2], F32, kind="ExternalInput").ap()
    prot_d = nc.dram_tensor("prot", [P, P], F32R, kind="ExternalInput").ap()
    ident_d = nc.dram_tensor("ident", [P, P], F32R, kind="ExternalInput").ap()
    esel_d = nc.dram_tensor("esel", [P, 5, 5], F32R,
                            kind="ExternalInput").ap()
    out_d = nc.dram_tensor("out", [SC, P, TC, 512], F32,
                           kind="ExternalOutput").ap()

    with tile.TileContext(nc) as tc:
        with (
            tc.tile_pool(name="consts", bufs=1) as cp,
        ):
            dramp = tc.alloc_tile_pool(name="dram_scratch", bufs=1,
                                       space="DRAM")
            rfac_dr = dramp.tile([5, S], F32, name="rfac_dr")
            rd_dr = [dramp.tile([4, 1024], F32, name=f"rd_dr{i}")
                     for i in range(2)]
            # stage-scoped persistent tensors (manual release for SBUF reuse)
            p1 = tc.alloc_tile_pool(name="p1", bufs=1)   # lives A..C
            qkv_raw = p1.tile([P, 5, S], F32, name="qkv_raw")     # 40KB
            vT_sb = p1.tile([P, S], F32R, name="vT_sb")           # 8KB
            rfac = p1.tile([5, S], F32, name="rfac")

            cos_sb = cp.tile([HEAD_DIM, S], F32, name="cos_sb")
            sin_sb = cp.tile([HEAD_DIM, S], F32, name="sin_sb")
            normw_sb = cp.tile([P, 2], F32, name="normw_sb")
            prot_sb = cp.tile([P, P], F32R, name="prot_sb")
            ident_sb = cp.tile([P, P], F32R, name="ident_sb")
            esel_sb = cp.tile([P, 5, 5], F32R, name="esel_sb")
            eps_sb = cp.tile([P, 1], F32, name="eps_sb")
            zero_sb = cp.tile([P, 1], F32, name="zero_sb")
            nc.vector.memset(eps_sb[:], EPS)
            nc.vector.memset(zero_sb[:], 0.0)
            nc.sync.dma_start(esel_sb[:], esel_d)

            # ---------------- Stage A: qkv projection + squares ----------
            # fc-outer so each weight chunk (lhsT) is reused across the
            # 4 token chunks (amortizes LDWEIGHTS 4x); full xT resident.
            with (
                tc.tile_pool(name="stA", bufs=2) as sa,
                tc.tile_pool(name="wq_pool", bufs=1) as wp,
                tc.tile_pool(name="psA", bufs=4, space="PSUM") as psA,
                tc.tile_pool(name="psSq", bufs=1, space="PSUM") as psSq,
            ):
                wq_sb = wp.tile([P, KC, NF], F32R, name="wq_sb")  # 48KB
                ssq_ps = psSq.tile([5, S], F32, name="ssq_ps")    # 4 banks
                for tcc in range(TC):
                    xt = sa.tile([P, KC, 512], F32R, name="xt")   # 32KB x2
                    step = 2 if tcc == 0 else 4
                    for kc4 in range(0, KC, step):
                        nc.sync.dma_start(xt[:, kc4:kc4 + step, :],
                                          xT_d[tcc, :, kc4:kc4 + step, :])
                        if tcc == 0:
                            nc.sync.dma_start(wq_sb[:, kc4:kc4 + step, :],
                                              wqkvT_d[:, kc4:kc4 + step, :])
                    for fc in range(FC):
                        ps = psA.tile([P, 512], F32, name="qkv_ps")
                        for kc in range(KC):
                            nc.tensor.matmul(
                                ps[:],
                                wq_sb[:, kc, fc * P:(fc + 1) * P],
                                xt[:, kc, :],
                                start=(kc == 0), stop=(kc == KC - 1))
                        tsl = slice(tcc * 512, (tcc + 1) * 512)
                        if fc < 5:
                            sq = sa.tile([P, 512], F32R, name="sq")
                            nc.scalar.activation(sq[:], ps[:], AF.Square,
                                                 bias=zero_sb[:])
                            nc.tensor.matmul(
                                ssq_ps[:, tsl], esel_sb[:, fc, :],
                                sq[:],
                                start=(fc == 0), stop=(fc == 4),
                                skip_group_check=True)
                            nc.vector.tensor_copy(qkv_raw[:, fc, tsl], ps[:])
                        else:
                            nc.vector.tensor_copy(vT_sb[:, tsl], ps[:])
                nc.sync.dma_start(cos_sb[:], cosT_d)
                nc.sync.dma_start(sin_sb[:], sinT_d)
                nc.sync.dma_start(normw_sb[:], normw_d)
                nc.sync.dma_start(prot_sb[:], prot_d)
                nc.sync.dma_start(ident_sb[:], ident_d)
                # Stage B: rms factors  rfac = 1/sqrt(ssq/128 + eps)
                std = wp.tile([5, S], F32, name="std")
                nc.scalar.activation(std[:], ssq_ps[:], AF.Sqrt,
                                     scale=1.0 / HEAD_DIM, bias=eps_sb[0:5, :])
                nc.vector.reciprocal_approx_fast(rfac[:], std[:])
                nc.gpsimd.dma_start(rfac_dr[:], rfac[:])

            # ---------------- Stage C: normalize + rope + v transpose ----
            p2 = tc.alloc_tile_pool(name="p2", bufs=1, side="right")  # C..D
            qk_sb = [p2.tile([P, S], F32R, name=f"qk_sb{i}")
                     for i in range(5)]                           # 40KB
            v_sb = p2.tile([P, SC, HEAD_DIM], F32R, name="v_sb")  # 8KB
            # scores psum allocated BEFORE stage C so its banks are not
            # WAR-blocked on C's rope/transpose psum (lets stage D start
            # while C's DVE tail is still running)
            psS = tc.alloc_tile_pool(name="psS", bufs=4, space="PSUM")
            with (
                tc.tile_pool(name="stC", bufs=2) as sc_pool,
                tc.tile_pool(name="psC", bufs=2, space="PSUM") as psC,
            ):
                for fc in (4, 0, 1, 2, 3):
                    rb = sc_pool.tile([P, S], F32, name="rb")
                    nc.gpsimd.dma_start(
                        rb[:], rfac_dr[fc:fc + 1, :].to_broadcast((P, S)))
                    qn = sc_pool.tile([P, S], F32R, name="qn")
                    wcol = 0 if fc < 4 else 1
                    nc.vector.scalar_tensor_tensor(
                        qn[:], qkv_raw[:, fc, :],
                        normw_sb[:, wcol:wcol + 1], rb[:],
                        op0=OP.mult, op1=OP.mult)
                    for tcc in range(TC):
                        tsl = slice(tcc * 512, (tcc + 1) * 512)
                        rot_ps = psC.tile([P, 512], F32, name="rot_ps")
                        nc.tensor.matmul(rot_ps[:], prot_sb[:], qn[:, tsl],
                                         start=True, stop=True)
                        rs = sc_pool.tile([P, 512], F32, name="rs")
                        nc.vector.tensor_mul(rs[:], rot_ps[:], sin_sb[:, tsl])
                        qc = sc_pool.tile([P, 512], F32, name="qc")
                        nc.gpsimd.tensor_mul(qc[:], qn[:, tsl].bitcast(F32),
                                             cos_sb[:, tsl])
                        nc.vector.tensor_add(qk_sb[fc][:, tsl], qc[:],
                                             rs[:])
                for scc in range(SC):
                    vt_ps = psC.tile([P, P], F32R, name="vt_ps")
                    nc.tensor.transpose(
                        vt_ps[:], vT_sb[:, scc * P:(scc + 1) * P],
                        ident_sb[:])
                    nc.vector.tensor_copy(v_sb[:, scc, :], vt_ps[:])

            p1.release()

            # ---------------- Stage D: attention ------------------------
            p3 = tc.alloc_tile_pool(name="p3", bufs=1)   # lives D..E
            atn_raw = [p3.tile([P, GQ, 1024], F32, name=f"atn_raw{i}")
                       for i in range(2)]                         # 32KB
            atn_n = [p3.tile([P, GQ, 1024], F32R, name=f"atn_n{i}")
                     for i in range(2)]                           # 32KB
            woT_sb = p3.tile([P, GQ, DIM], F32R, name="woT_sb")   # 32KB
            with (
                tc.tile_pool(name="stD", bufs=2) as sd,
                tc.tile_pool(name="ptp", bufs=6) as ptp,
                tc.tile_pool(name="psPV", bufs=1, space="PSUM") as psPV,
                tc.tile_pool(name="psDN", bufs=1, space="PSUM") as psDN,
            ):
                for pair in range(2):
                    po = pair * 1024
                    dn_ps = psDN.tile([4, 1024], F32, name="dn_ps")
                    for h in range(GQ):
                        pv_ps = psPV.tile([P, 1024], F32, name="pv_ps")
                        for scc in range(SC):
                            ksl = qk_sb[4][:, scc * P:(scc + 1) * P]
                            for half in range(2):
                                hs = slice(half * 512, half * 512 + 512)
                                sp = psS.tile([P, 512], F32, name="sp")
                                nc.tensor.matmul(
                                    sp[:], ksl,
                                    qk_sb[h][:, po + half * 512:
                                              po + half * 512 + 512],
                                    start=True, stop=True)
                                pt = ptp.tile([P, 512], F32R, name="pt")
                                nc.scalar.activation(pt[:], sp[:], AF.Exp,
                                                     bias=zero_sb[:])
                                nc.tensor.matmul(
                                    pv_ps[:, hs], v_sb[:, scc, :], pt[:],
                                    start=(scc == 0), stop=(scc == SC - 1))
                                nc.tensor.matmul(
                                    dn_ps[:, hs], esel_sb[:, h, 0:4],
                                    pt[:],
                                    start=(h == 0 and scc == 0),
                                    stop=(h == GQ - 1 and scc == SC - 1),
                                    skip_group_check=True)
                        nc.vector.tensor_copy(atn_raw[pair][:, h, :],
                                              pv_ps[:])
                    rd = sd.tile([4, 1024], F32, name="rd")
                    nc.vector.reciprocal_approx_fast(rd[:], dn_ps[:])
                    nc.gpsimd.dma_start(rd_dr[pair][:], rd[:])
                    for h in range(GQ):
                        rbh = sd.tile([P, 1024], F32, name="rbh")
                        nc.gpsimd.dma_start(
                            rbh[:],
                            rd_dr[pair][h:h + 1, :].to_broadcast((P, 1024)))
                        nc.vector.tensor_mul(atn_n[pair][:, h, :],
                                             atn_raw[pair][:, h, :],
                                             rbh[:])

            psS.release()
            p2.release()

            # ---------------- Stage E: output projection -----------------
            nc.sync.dma_start(woT_sb[:], woT_d)
            with (
                tc.tile_pool(name="stE", bufs=4) as se,
                tc.tile_pool(name="psE", bufs=8, space="PSUM") as psE,
            ):
                for tcc in range(SC):
                    pr = tcc // 8
                    tloc = (tcc % 8) * P
                    ps4 = [psE.tile([P, 512], F32, name="out_ps")
                           for _ in range(TC)]
                    for h in range(GQ):
                        lhs = atn_n[pr][:, h, tloc:tloc + P]
                        for oc in range(TC):
                            nc.tensor.matmul(
                                ps4[oc][:], lhs,
                                woT_sb[:, h, oc * 512:(oc + 1) * 512],
                                start=(h == 0), stop=(h == GQ - 1),
                                skip_group_check=True)
                    ob = se.tile([P, TC, 512], F32, name="ob")
                    for oc in range(TC):
                        nc.vector.tensor_copy(ob[:, oc, :], ps4[oc][:])
                    nc.sync.dma_start(out_d[tcc], ob[:])
            p3.release()

    nc.compile()
    return nc


def make_in_maps(x, wqkv, wo, q_norm_w, k_norm_w, freqs_cos, freqs_sin):
    """Build the 8 per-core input maps. Core c = b*4 + g."""
    x = np.asarray(x, np.float32)
    wqkv = np.asarray(wqkv, np.float32)
    wo = np.asarray(wo, np.float32)
    q_norm_w = np.asarray(q_norm_w, np.float32)
    k_norm_w = np.asarray(k_norm_w, np.float32)
    cosT = np.ascontiguousarray(
        np.asarray(freqs_cos, np.float32)[:, 0, :].T)
    sinT = np.ascontiguousarray(
        np.asarray(freqs_sin, np.float32)[:, 0, :].T)

    normw = np.empty((P, 2), np.float32)
    normw[:, 0] = q_norm_w * np.float32(1.0 / np.sqrt(HEAD_DIM))
    normw[:, 1] = k_norm_w

    prot = np.zeros((P, P), np.float32)
    prot[np.arange(1, P, 2), np.arange(0, P, 2)] = -1.0
    prot[np.arange(0, P, 2), np.arange(1, P, 2)] = 1.0
    ident = np.eye(P, dtype=np.float32)
    esel = np.zeros((P, 5, 5), np.float32)
    for c in range(5):
        esel[:, c, c] = 1.0

    q_size = N_HEADS * HEAD_DIM
    kv_size = N_KV * HEAD_DIM
    in_maps = []
    for b in range(B):
        # [tc, p, kc, u]: xT[kc*128+p, tc*512+u] pre-tiled for 1-run/partition
        xT = np.ascontiguousarray(
            x[b].reshape(TC, 512, KC, P).transpose(0, 3, 2, 1))
        for g in range(N_KV):
            wq = wqkv[g * GF:(g + 1) * GF]
            wk = wqkv[q_size + g * HEAD_DIM:q_size + (g + 1) * HEAD_DIM]
            wv = wqkv[q_size + kv_size + g * HEAD_DIM:
                      q_size + kv_size + (g + 1) * HEAD_DIM]
            wqkvT = np.ascontiguousarray(
                np.concatenate([wq, wk, wv], axis=0).T
                .reshape(KC, P, NF).transpose(1, 0, 2))
            woT = np.ascontiguousarray(
                wo[:, g * GF:(g + 1) * GF].T.reshape(GQ, HEAD_DIM, DIM)
                .transpose(1, 0, 2))
            in_maps.append({
                "xT": xT, "wqkvT": wqkvT, "woT": woT,
                "cosT": cosT, "sinT": sinT, "normw": normw,
                "prot": prot, "ident": ident, "esel": esel,
            })
    return in_maps


def run(in_maps, trace=False):
    global _CACHED_NC
    if _CACHED_NC is None:
        _CACHED_NC = build_nc()
    return bass_utils.run_bass_kernel_spmd(
        _CACHED_NC, in_maps, core_ids=list(range(8)), trace=trace)


def kernel(x, wqkv, wo, q_norm_w, k_norm_w, freqs_cos, freqs_sin):
    in_maps = make_in_maps(x, wqkv, wo, q_norm_w, k_norm_w,
                           freqs_cos, freqs_sin)
    res = run(in_maps, trace=False)
    out = np.zeros((B, S, DIM), np.float32)
    for b in range(B):
        for g in range(N_KV):
            o = res.results[b * N_KV + g]["out"]    # [SC, P, TC, 512]
            out[b] += o.transpose(0, 1, 2, 3).reshape(S, DIM)
    return out


# revision 26
# speedup vs baseline: 1.0490x; 1.0194x over previous
"""Trainium2 Bass kernel for nn_Attention_3710851743764.

Full attention block: qkv proj -> per-head RMSNorm(q,k) -> RoPE -> GQA
attention (16 q heads, 4 kv heads, S=2048, D=128) -> out proj.

Sharding: 8 cores = 2 (batch) x 4 (kv-head groups). Each core computes its
batch's qkv for its group (4 q heads + 1 kv head), full attention for those
heads, and a partial output projection (its 512 wo columns); the host sums
the 4 partials per batch.

Dataflow is fully "transposed" (features on partitions, tokens on free):
  qkvT[f,t]   = mm(lhsT=wqkvT[d,f], rhs=xT[d,t])            accumulated over d
  ssq[c,t]    = mm(lhsT=esel[:,c,:], rhs=square(qkvT_c))     (RMS factors)
  rot[d',t]   = mm(lhsT=P_rot, rhs=qn)                       (RoPE pair swap)
  scoresT[s,t]= mm(lhsT=kT[:,s-blk], rhs=qT_h)               per 128-s block
  pT          = exp(scoresT)          (no max subtraction: |score|<=sqrt(128))
  attnT[d,t]  = mm(lhsT=v[s-blk,d], rhs=pT)                  accumulated over s
  denom[h,t]  = mm(lhsT=esel[:,h,:4], rhs=pT)                accumulated
  out[t,o]    = mm(lhsT=attnT_n[f,t-blk], rhs=woT[f,o])      accumulated over f

All matmuls run in float32r (tf32-like, full PE rate).
"""

import sys

sys.path.insert(0, "/opt/trn_rl_repo")

import numpy as np

import concourse.bass as bass
import concourse.tile as tile
from concourse import bacc, mybir
from concourse import bass_utils

F32 = mybir.dt.float32
F32R = mybir.dt.float32r
AF = mybir.ActivationFunctionType
OP = mybir.AluOpType

DIM = 2048
N_HEADS = 16
N_KV = 4
HEAD_DIM = 128
B = 2
S = 2048
EPS = float(np.finfo(np.float32).eps)
GQ = N_HEADS // N_KV          # q heads per group = 4
GF = GQ * HEAD_DIM            # group q features = 512
P = 128
KC = DIM // P                 # 16 contraction chunks for projections
TC = 4                        # token chunks of 512
SC = S // P                   # 16 key chunks of 128
NF = GF + 2 * HEAD_DIM        # 768 qkv features per group
FC = NF // P                  # 6 feature chunks

_CACHED_NC = None


def build_nc():
    """Build the single-core Bass program (same program for all 8 cores)."""
    nc = bacc.Bacc("TRN2", target_bir_lowering=False, debug=False,
                   num_devices=8)

    xT_d = nc.dram_tensor("xT", [TC, P, KC, 512], F32R,
                          kind="ExternalInput").ap()
    wqkvT_d = nc.dram_tensor("wqkvT", [P, KC, NF], F32R,
                             kind="ExternalInput").ap()
    woT_d = nc.dram_tensor("woT", [HEAD_DIM, GQ, DIM], F32R,
                           kind="ExternalInput").ap()
    cosT_d = nc.dram_tensor("cosT", [HEAD_DIM, S], F32,
                            kind="ExternalInput").ap()
    sinT_d = nc.dram_tensor("sinT", [HEAD_DIM, S], F32,
                            kind="ExternalInput").ap()
    normw_d = nc.dram_tensor("normw", [P, 2], F32, kind="ExternalInput").ap()
    prot_d = nc.dram_tensor("prot", [P, P], F32R, kind="ExternalInput").ap()
    ident_d = nc.dram_tensor("ident", [P, P], F32R, kind="ExternalInput").ap()
    esel_d = nc.dram_tensor("esel", [P, 5, 5], F32R,
                            kind="ExternalInput").ap()
    out_d = nc.dram_tensor("out", [SC, P, TC, 512], F32,
                           kind="ExternalOutput").ap()

    with tile.TileContext(nc) as tc:
        with (
            tc.tile_pool(name="consts", bufs=1) as cp,
        ):
            dramp = tc.alloc_tile_pool(name="dram_scratch", bufs=1,
                                       space="DRAM")
            rfac_dr = dramp.tile([5, S], F32, name="rfac_dr")
            rd_dr = [dramp.tile([4, 1024], F32, name=f"rd_dr{i}")
                     for i in range(2)]
            # stage-scoped persistent tensors (manual release for SBUF reuse)
            p1 = tc.alloc_tile_pool(name="p1", bufs=1)   # lives A..C
            qkv_raw = p1.tile([P, 5, S], F32, name="qkv_raw")     # 40KB
            vT_sb = p1.tile([P, S], F32R, name="vT_sb")           # 8KB
            rfac = p1.tile([5, S], F32, name="rfac")

            cos_sb = cp.tile([HEAD_DIM, S], F32, name="cos_sb")
            sin_sb = cp.tile([HEAD_DIM, S], F32, name="sin_sb")
            normw_sb = cp.tile([P, 2], F32, name="normw_sb")
            prot_sb = cp.tile([P, P], F32R, name="prot_sb")
            ident_sb = cp.tile([P, P], F32R, name="ident_sb")
            esel_sb = cp.tile([P, 5, 5], F32R, name="esel_sb")
            eps_sb = cp.tile([P, 1], F32, name="eps_sb")
            zero_sb = cp.tile([P, 1], F32, name="zero_sb")
            nc.vector.memset(eps_sb[:], EPS)
            nc.vector.memset(zero_sb[:], 0.0)
            nc.sync.dma_start(esel_sb[:], esel_d)

            # ---------------- Stage A: qkv projection + squares ----------
            # fc-outer so each weight chunk (lhsT) is reused across the
            # 4 token chunks (amortizes LDWEIGHTS 4x); full xT resident.
            with (
                tc.tile_pool(name="stA", bufs=2) as sa,
                tc.tile_pool(name="wq_pool", bufs=1) as wp,
                tc.tile_pool(name="psA", bufs=4, space="PSUM") as psA,
                tc.tile_pool(name="psSq", bufs=1, space="PSUM") as psSq,
            ):
                wq_sb = wp.tile([P, KC, NF], F32R, name="wq_sb")  # 48KB
                ssq_ps = psSq.tile([5, S], F32, name="ssq_ps")    # 4 banks
                for tcc in range(TC):
                    xt = sa.tile([P, KC, 512], F32R, name="xt")   # 32KB x2
                    for kc4 in range(0, KC, 4):
                        nc.sync.dma_start(xt[:, kc4:kc4 + 4, :],
                                          xT_d[tcc, :, kc4:kc4 + 4, :])
                        if tcc == 0:
                            nc.sync.dma_start(wq_sb[:, kc4:kc4 + 4, :],
                                              wqkvT_d[:, kc4:kc4 + 4, :])
                    for fc in range(FC):
                        ps = psA.tile([P, 512], F32, name="qkv_ps")
                        for kc in range(KC):
                            nc.tensor.matmul(
                                ps[:],
                                wq_sb[:, kc, fc * P:(fc + 1) * P],
                                xt[:, kc, :],
                                start=(kc == 0), stop=(kc == KC - 1))
                        tsl = slice(tcc * 512, (tcc + 1) * 512)
                        if fc < 5:
                            sq = sa.tile([P, 512], F32R, name="sq")
                            nc.scalar.activation(sq[:], ps[:], AF.Square,
                                                 bias=zero_sb[:])
                            nc.tensor.matmul(
                                ssq_ps[:, tsl], esel_sb[:, fc, :],
                                sq[:],
                                start=(fc == 0), stop=(fc == 4),
                                skip_group_check=True)
                            nc.vector.tensor_copy(qkv_raw[:, fc, tsl], ps[:])
                        else:
                            nc.vector.tensor_copy(vT_sb[:, tsl], ps[:])
                nc.sync.dma_start(cos_sb[:], cosT_d)
                nc.sync.dma_start(sin_sb[:], sinT_d)
                nc.sync.dma_start(normw_sb[:], normw_d)
                nc.sync.dma_start(prot_sb[:], prot_d)
                nc.sync.dma_start(ident_sb[:], ident_d)
                # Stage B: rms factors  rfac = 1/sqrt(ssq/128 + eps)
                std = wp.tile([5, S], F32, name="std")
                nc.scalar.activation(std[:], ssq_ps[:], AF.Sqrt,
                                     scale=1.0 / HEAD_DIM, bias=eps_sb[0:5, :])
                nc.vector.reciprocal_approx_fast(rfac[:], std[:])
                nc.gpsimd.dma_start(rfac_dr[:], rfac[:])

            # ---------------- Stage C: normalize + rope + v transpose ----
            p2 = tc.alloc_tile_pool(name="p2", bufs=1, side="right")  # C..D
            qk_sb = [p2.tile([P, S], F32R, name=f"qk_sb{i}")
                     for i in range(5)]                           # 40KB
            v_sb = p2.tile([P, SC, HEAD_DIM], F32R, name="v_sb")  # 8KB
            # scores psum allocated BEFORE stage C so its banks are not
            # WAR-blocked on C's rope/transpose psum (lets stage D start
            # while C's DVE tail is still running)
            psS = tc.alloc_tile_pool(name="psS", bufs=4, space="PSUM")
            with (
                tc.tile_pool(name="stC", bufs=2) as sc_pool,
                tc.tile_pool(name="psC", bufs=2, space="PSUM") as psC,
            ):
                for fc in (4, 0, 1, 2, 3):
                    rb = sc_pool.tile([P, S], F32, name="rb")
                    nc.gpsimd.dma_start(
                        rb[:], rfac_dr[fc:fc + 1, :].to_broadcast((P, S)))
                    qn = sc_pool.tile([P, S], F32R, name="qn")
                    wcol = 0 if fc < 4 else 1
                    nc.vector.scalar_tensor_tensor(
                        qn[:], qkv_raw[:, fc, :],
                        normw_sb[:, wcol:wcol + 1], rb[:],
                        op0=OP.mult, op1=OP.mult)
                    for tcc in range(TC):
                        tsl = slice(tcc * 512, (tcc + 1) * 512)
                        rot_ps = psC.tile([P, 512], F32, name="rot_ps")
                        nc.tensor.matmul(rot_ps[:], prot_sb[:], qn[:, tsl],
                                         start=True, stop=True)
                        rs = sc_pool.tile([P, 512], F32, name="rs")
                        nc.vector.tensor_mul(rs[:], rot_ps[:], sin_sb[:, tsl])
                        qc = sc_pool.tile([P, 512], F32, name="qc")
                        nc.gpsimd.tensor_mul(qc[:], qn[:, tsl].bitcast(F32),
                                             cos_sb[:, tsl])
                        nc.vector.tensor_add(qk_sb[fc][:, tsl], qc[:],
                                             rs[:])
                for scc in range(SC):
                    vt_ps = psC.tile([P, P], F32R, name="vt_ps")
                    nc.tensor.transpose(
                        vt_ps[:], vT_sb[:, scc * P:(scc + 1) * P],
                        ident_sb[:])
                    nc.vector.tensor_copy(v_sb[:, scc, :], vt_ps[:])

            p1.release()

            # ---------------- Stage D: attention ------------------------
            p3 = tc.alloc_tile_pool(name="p3", bufs=1)   # lives D..E
            atn_raw = [p3.tile([P, GQ, 1024], F32, name=f"atn_raw{i}")
                       for i in range(2)]                         # 32KB
            atn_n = [p3.tile([P, GQ, 1024], F32R, name=f"atn_n{i}")
                     for i in range(2)]                           # 32KB
            woT_sb = p3.tile([P, GQ, DIM], F32R, name="woT_sb")   # 32KB
            with (
                tc.tile_pool(name="stD", bufs=2) as sd,
                tc.tile_pool(name="ptp", bufs=6) as ptp,
                tc.tile_pool(name="psPV", bufs=1, space="PSUM") as psPV,
                tc.tile_pool(name="psDN", bufs=1, space="PSUM") as psDN,
            ):
                for pair in range(2):
                    po = pair * 1024
                    dn_ps = psDN.tile([4, 1024], F32, name="dn_ps")
                    for h in range(GQ):
                        pv_ps = psPV.tile([P, 1024], F32, name="pv_ps")
                        for scc in range(SC):
                            ksl = qk_sb[4][:, scc * P:(scc + 1) * P]
                            for half in range(2):
                                hs = slice(half * 512, half * 512 + 512)
                                sp = psS.tile([P, 512], F32, name="sp")
                                nc.tensor.matmul(
                                    sp[:], ksl,
                                    qk_sb[h][:, po + half * 512:
                                              po + half * 512 + 512],
                                    start=True, stop=True)
                                pt = ptp.tile([P, 512], F32R, name="pt")
                                nc.scalar.activation(pt[:], sp[:], AF.Exp,
                                                     bias=zero_sb[:])
                                nc.tensor.matmul(
                                    pv_ps[:, hs], v_sb[:, scc, :], pt[:],
                                    start=(scc == 0), stop=(scc == SC - 1))
                                nc.tensor.matmul(
                                    dn_ps[:, hs], esel_sb[:, h, 0:4],
                                    pt[:],
                                    start=(h == 0 and scc == 0),
                                    stop=(h == GQ - 1 and scc == SC - 1),
                                    skip_group_check=True)
                        nc.vector.tensor_copy(atn_raw[pair][:, h, :],
                                              pv_ps[:])
                    rd = sd.tile([4, 1024], F32, name="rd")
                    nc.vector.reciprocal_approx_fast(rd[:], dn_ps[:])
                    nc.gpsimd.dma_start(rd_dr[pair][:], rd[:])
                    for h in range(GQ):
                        rbh = sd.tile([P, 1024], F32, name="rbh")
                        nc.gpsimd.dma_start(
                            rbh[:],
                            rd_dr[pair][h:h + 1, :].to_broadcast((P, 1024)))
                        nc.vector.tensor_mul(atn_n[pair][:, h, :],
                                             atn_raw[pair][:, h, :],
                                             rbh[:])

            psS.release()
            p2.release()

            # ---------------- Stage E: output projection -----------------
            nc.sync.dma_start(woT_sb[:], woT_d)
            with (
                tc.tile_pool(name="stE", bufs=4) as se,
                tc.tile_pool(name="psE", bufs=8, space="PSUM") as psE,
            ):
                for tcc in range(SC):
                    pr = tcc // 8
                    tloc = (tcc % 8) * P
                    ps4 = [psE.tile([P, 512], F32, name="out_ps")
                           for _ in range(TC)]
                    for h in range(GQ):
                        lhs = atn_n[pr][:, h, tloc:tloc + P]
                        for oc in range(TC):
                            nc.tensor.matmul(
                                ps4[oc][:], lhs,
                                woT_sb[:, h, oc * 512:(oc + 1) * 512],
                                start=(h == 0), stop=(h == GQ - 1),
                                skip_group_check=True)
                    ob = se.tile([P, TC, 512], F32, name="ob")
                    for oc in range(TC):
                        nc.vector.tensor_copy(ob[:, oc, :], ps4[oc][:])
                    nc.sync.dma_start(out_d[tcc], ob[:])
            p3.release()

    nc.compile()
    return nc


def make_in_maps(x, wqkv, wo, q_norm_w, k_norm_w, freqs_cos, freqs_sin):
    """Build the 8 per-core input maps. Core c = b*4 + g."""
    x = np.asarray(x, np.float32)
    wqkv = np.asarray(wqkv, np.float32)
    wo = np.asarray(wo, np.float32)
    q_norm_w = np.asarray(q_norm_w, np.float32)
    k_norm_w = np.asarray(k_norm_w, np.float32)
    cosT = np.ascontiguousarray(
        np.asarray(freqs_cos, np.float32)[:, 0, :].T)
    sinT = np.ascontiguousarray(
        np.asarray(freqs_sin, np.float32)[:, 0, :].T)

    normw = np.empty((P, 2), np.float32)
    normw[:, 0] = q_norm_w * np.float32(1.0 / np.sqrt(HEAD_DIM))
    normw[:, 1] = k_norm_w

    prot = np.zeros((P, P), np.float32)
    prot[np.arange(1, P, 2), np.arange(0, P, 2)] = -1.0
    prot[np.arange(0, P, 2), np.arange(1, P, 2)] = 1.0
    ident = np.eye(P, dtype=np.float32)
    esel = np.zeros((P, 5, 5), np.float32)
    for c in range(5):
        esel[:, c, c] = 1.0

    q_size = N_HEADS * HEAD_DIM
    kv_size = N_KV * HEAD_DIM
    in_maps = []
    for b in range(B):
        # [tc, p, kc, u]: xT[kc*128+p, tc*512+u] pre-tiled for 1-run/partition
        xT = np.ascontiguousarray(
            x[b].reshape(TC, 512, KC, P).transpose(0, 3, 2, 1))
        for g in range(N_KV):
            wq = wqkv[g * GF:(g + 1) * GF]
            wk = wqkv[q_size + g * HEAD_DIM:q_size + (g + 1) * HEAD_DIM]
            wv = wqkv[q_size + kv_size + g * HEAD_DIM:
                      q_size + kv_size + (g + 1) * HEAD_DIM]
            wqkvT = np.ascontiguousarray(
                np.concatenate([wq, wk, wv], axis=0).T
                .reshape(KC, P, NF).transpose(1, 0, 2))
            woT = np.ascontiguousarray(
                wo[:, g * GF:(g + 1) * GF].T.reshape(GQ, HEAD_DIM, DIM)
                .transpose(1, 0, 2))
            in_maps.append({
                "xT": xT, "wqkvT": wqkvT, "woT": woT,
                "cosT": cosT, "sinT": sinT, "normw": normw,
                "prot": prot, "ident": ident, "esel": esel,
            })
    return in_maps


def run(in_maps, trace=False):
    global _CACHED_NC
    if _CACHED_NC is None:
        _CACHED_NC = build_nc()
    return bass_utils.run_bass_kernel_spmd(
        _CACHED_NC, in_maps, core_ids=list(range(8)), trace=trace)


def kernel(x, wqkv, wo, q_norm_w, k_norm_w, freqs_cos, freqs_sin):
    in_maps = make_in_maps(x, wqkv, wo, q_norm_w, k_norm_w,
                           freqs_cos, freqs_sin)
    res = run(in_maps, trace=False)
    out = np.zeros((B, S, DIM), np.float32)
    for b in range(B):
        for g in range(N_KV):
            o = res.results[b * N_KV + g]["out"]    # [SC, P, TC, 512]
            out[b] += o.transpose(0, 1, 2, 3).reshape(S, DIM)
    return out
